# revision 1
# baseline (speedup 1.0000x reference)
"""Bass/Trainium2 kernel for nn_BoundaryLoss (8-core data-parallel).

loss = mean( ce * weight ) over (B=16, H=360, W=640) pixels, where
  ce     = logsumexp_c(pred) - pred[target]          (C=7)
  weight = 10 if 5x5-ellipse window around the pixel is NOT constant else 1
           (morphological gradient > 0, cv2 border-ignoring semantics)

Sharding: pure data parallel, 2 images per NeuronCore.  Each core emits a
[128, 64] f32 accumulator tile holding per-partition partial sums
(w = 1 + 9*boundary):
  cols  0..11 : sum(w * lse)     per (group, half)
  cols 32..43 : sum(w * picked)  per (group, half)
Host: loss = ( S_wlse - S_wpk ) / (B*H*W)  -- the tiny 8-way combine is the
all-reduce from the sharding hint, done on host since kernel() returns the
full output anyway.

Morphology is computed exactly with the variance trick: the window is
constant  <=>  17*S2 == S1^2  where S1 = sum(t), S2 = sum(t^2) over the
17-tap ellipse with replicate clamping at borders (replicate-clamped taps
always fall inside the in-image window, so this matches cv2's
border-ignoring max/min).  All quantities are small integers -> exact in
bf16 matmuls + fp32 PSUM.
"""

import sys

for _p in ("/opt/trn_rl_repo",):
    if _p not in sys.path:
        sys.path.insert(0, _p)

import numpy as np
import ml_dtypes

import bass_rust
import concourse.bass as bass
import concourse.mybir as mybir
from concourse.tile import TileContext
from concourse import bass_utils

F32 = mybir.dt.float32
BF16 = mybir.dt.float16  # fp16: 10-bit mantissa, exact ints 0..2048, exp(P)<=~200 safe
I32 = mybir.dt.int32

B_PER_CORE = 2
H, W, C = 360, 640, 7
# (row0, rows, variant): variant 0=top-clamped, 1=interior, 2=bottom-clamped
GROUPS = [(0, 124, 0), (124, 124, 1), (248, 112, 2)]
NV = 3  # conv variants stored in convw
WPAD = W + 4
NCOL = 64  # acc tile columns

# ellipse 5x5 taps grouped by dx -> vertical dy list
VERT = {0: [-2, -1, 0, 1, 2], -1: [-1, 0, 1], 1: [-1, 0, 1],
        -2: [-1, 0, 1], 2: [-1, 0, 1]}
DXS = [-2, -1, 0, 1, 2]


def _build_convw():
    """[16, 128, 124] bf16: per group-position (3) x dx (5) banded vertical
    conv lhsT with border clamping baked in; slot 15 = identity."""
    w = np.zeros((16, 128, 124), dtype=np.float32)
    seen = {}
    for (r0, R, v) in GROUPS:
        if v in seen:
            continue
        seen[v] = True
        in_r0 = max(r0 - 2, 0)
        in_r1 = min(r0 + R + 2, H)
        for dxi, dx in enumerate(DXS):
            for j in range(R):
                for dy in VERT[dx]:
                    rr = min(max(r0 + j + dy, 0), H - 1)
                    k = rr - in_r0
                    assert 0 <= k < in_r1 - in_r0 <= 128
                    w[v * 5 + dxi, k, j] += 1.0
    for k in range(124):
        w[15, k, k] = 1.0
    return np.ascontiguousarray(
        w.transpose(1, 0, 2).reshape(128, 16 * 124)).astype(np.float16)


def split_multiwait_drains(nc, max_waits=1):
    """This walrus build rejects >1 sync-waits on CTRL-class instructions
    (the Tile end-of-kernel drain).  Split extra waits into preceding
    single-wait EventSemaphore instructions on the same engine."""
    fn = nc.m.functions[0]
    for bb in fn.blocks:
        for inst in list(bb.instructions):
            si = inst.sync_info
            if si is None or len(si.on_wait) <= max_waits:
                continue
            waits = list(si.on_wait)
            keep, extra = waits[:max_waits], waits[max_waits:]
            new_insts = []
            for k, wt in enumerate(extra):
                es = mybir.InstEventSemaphore(
                    name=f"{inst.name}-waitsplit-{k}", ins=[], outs=[])
                es.engine = inst.engine
                es.sync_info = bass_rust.SyncInfo(on_wait=[wt], on_update=[])
                nc.register_instruction(es, overwrite=True)
                new_insts.append(es)
            inst.sync_info = bass_rust.SyncInfo(
                on_wait=keep, on_update=list(si.on_update))
            pos = [i.name for i in bb.instructions].index(inst.name)
            for k, es in enumerate(new_insts):
                bb.instructions.insert(pos + k, es)


def _emit_group(nc, tc, pools, aps, b, gi):
    """Emit all work for (image b, row-group gi)."""
    r0, R, var = GROUPS[gi]
    in_r0 = max(r0 - 2, 0)
    in_r1 = min(r0 + R + 2, H)
    n_in = in_r1 - in_r0
    g = b * len(GROUPS) + gi  # global group index

    pred, target, convw_sb, acc = aps[:4]
    io, sm, ps, psm = pools

    alu = mybir.AluOpType
    AF = mybir.ActivationFunctionType

    # ---- loads (t first: small DMAs unblock DVE/PE while P streams) ----
    t_pad = sm.tile([128, WPAD], BF16, tag="t_pad")
    nc.gpsimd.dma_start(out=t_pad[:n_in, 2:2 + W],
                        in_=target[b, in_r0:in_r1, :])
    if var == 0:
        # top group: rows start at partition 0 of t_pad, reuse it directly
        t_ctr = t_pad[:, 2:2 + W]
    else:
        t_ctr = sm.tile([128, W], BF16, tag="t_ctr")
        nc.gpsimd.dma_start(out=t_ctr[:R, :], in_=target[b, r0:r0 + R, :])

    P = io.tile([128, C * W], BF16, tag="P")
    nc.gpsimd.dma_start(
        out=P[:R, :],
        in_=pred[b, :, r0:r0 + R, :].rearrange("c r w -> r c w"))
    # horizontal replicate pad (2 cols each side)
    nc.vector.tensor_copy(t_pad[:n_in, 0:2],
                          t_pad[:n_in, 2:3].broadcast_to([n_in, 2]))
    nc.vector.tensor_copy(t_pad[:n_in, W + 2:W + 4],
                          t_pad[:n_in, W + 1:W + 2].broadcast_to([n_in, 2]))

    t2_pad = sm.tile([128, WPAD], BF16, tag="t2_pad")
    nc.scalar.square(t2_pad[:n_in, :], t_pad[:n_in, :])

    # ---- CE: mask / exp / reduce ----------------------------------------
    MG = io.tile([128, C * W], BF16, tag="MG")
    for c in range(C):
        sl = slice(c * W, (c + 1) * W)
        nc.vector.tensor_scalar(out=MG[:R, sl], in0=t_ctr[:R, :],
                                scalar1=float(c), scalar2=None,
                                op0=alu.is_equal)
    E = io.tile([128, C * W], BF16, tag="E")
    nc.scalar.activation(E[:R, :C * W // 2], P[:R, :C * W // 2], AF.Exp)
    nc.scalar.activation(E[:R, C * W // 2:], P[:R, C * W // 2:], AF.Exp)

    MP = io.tile([128, C * W], BF16, tag="MP")
    nc.vector.tensor_mul(MP[:R, :], MG[:R, :], P[:R, :])

    idw = convw_sb[:R, 15 * 124:15 * 124 + R]
    HW_ = W // 2  # 320-col halves: S/PK PSUM tiles are one bank each

    # morphology: S1/S2 ellipse conv on PE, full-width 2-bank PSUM tiles
    # (bufs=1 pool: their consumers below are fast, so serialization is
    # cheap, and full-width halves the fixed cost of square/cmp/W ops)
    S2_ps = psm.tile([128, W], F32, tag="S2")
    S1_ps = psm.tile([128, W], F32, tag="S1")
    for dxi, dx in enumerate(DXS):
        co = (var * 5 + dxi) * 124
        lhsT = convw_sb[:n_in, co:co + R]
        st, sp = (dxi == 0), (dxi == 4)
        for (c0, c1) in ((0, 512), (512, W)):
            nc.tensor.matmul(S2_ps[:R, c0:c1], lhsT,
                             t2_pad[:n_in, 2 + dx + c0:2 + dx + c1],
                             start=st, stop=sp)
            nc.tensor.matmul(S1_ps[:R, c0:c1], lhsT,
                             t_pad[:n_in, 2 + dx + c0:2 + dx + c1],
                             start=st, stop=sp)

    S1sq = sm.tile([128, W], F32, tag="S1sq")
    nc.scalar.square(S1sq[:R, :], S1_ps[:R, :])
    m = sm.tile([128, W], BF16, tag="m")
    nc.vector.scalar_tensor_tensor(
        out=m[:R, :], in0=S2_ps[:R, :], scalar=17.0, in1=S1sq[:R, :],
        op0=alu.mult, op1=alu.is_gt)
    Wt = sm.tile([128, W], BF16, tag="Wt")
    nc.vector.tensor_scalar(out=Wt[:R, :], in0=m[:R, :], scalar1=9.0,
                            scalar2=1.0, op0=alu.mult, op1=alu.add)

    for h in range(2):
        hs = slice(h * HW_, (h + 1) * HW_)
        S_ps = ps.tile([128, HW_], F32, tag="S")
        PK_ps = ps.tile([128, HW_], F32, tag="PK")
        for c in range(C):
            sl = slice(c * W + h * HW_, c * W + (h + 1) * HW_)
            st, sp = (c == 0), (c == C - 1)
            nc.tensor.matmul(S_ps[:R, :], idw, E[:R, sl], start=st, stop=sp)
            nc.tensor.matmul(PK_ps[:R, :], idw, MP[:R, sl], start=st, stop=sp)

        lse = sm.tile([128, HW_], BF16, tag="lse")
        nc.scalar.activation(lse[:R, :], S_ps[:R, :], AF.Ln)

        junk1 = sm.tile([128, HW_], BF16, tag="junk1")
        nc.vector.scalar_tensor_tensor(
            out=junk1[:R, :], in0=Wt[:R, hs], scalar=0.0, in1=lse[:R, :],
            op0=alu.bypass, op1=alu.mult,
            accum_out=acc[:R, 2 * g + h:2 * g + h + 1])
        junk2 = sm.tile([128, HW_], BF16, tag="junk2")
        nc.vector.scalar_tensor_tensor(
            out=junk2[:R, :], in0=Wt[:R, hs], scalar=0.0, in1=PK_ps[:R, :],
            op0=alu.bypass, op1=alu.mult,
            accum_out=acc[:R, 32 + 2 * g + h:32 + 2 * g + h + 1])


def build_nc(io_bufs=3, sm_bufs=7, ps_bufs=2, pool_mode="stack"):
    nc = bass.Bass()
    pred = nc.dram_tensor("pred", [B_PER_CORE, C, H, W], F32,
                          kind="ExternalInput")
    target = nc.dram_tensor("target", [B_PER_CORE, H, W], I32,
                            kind="ExternalInput")
    convw = nc.dram_tensor("convw", [128, 16 * 124], BF16,
                           kind="ExternalInput")
    acc_out = nc.dram_tensor("acc", [128, NCOL], F32, kind="ExternalOutput")

    with TileContext(nc, pool_alloc_mode=pool_mode) as tc:
        with (
            tc.tile_pool(name="io", bufs=io_bufs) as io,
            tc.tile_pool(name="sm", bufs=sm_bufs) as sm,
            tc.tile_pool(name="ps", bufs=ps_bufs, space="PSUM") as ps,
            tc.tile_pool(name="psm", bufs=1, space="PSUM") as psm,
            tc.tile_pool(name="const", bufs=1) as cpool,
        ):
            convw_sb = cpool.tile([128, 16 * 124], BF16)
            nc.sync.dma_start(out=convw_sb[:, :], in_=convw.ap())
            acc = cpool.tile([128, NCOL], F32)
            nc.vector.memset(acc[:, :], 0.0)
            aps = (pred.ap(), target.ap(), convw_sb, acc)
            for b in range(B_PER_CORE):
                for gi in range(len(GROUPS)):
                    _emit_group(nc, tc, (io, sm, ps, psm), aps, b, gi)

            nc.sync.dma_start(out=acc_out.ap(), in_=acc[:, :])

    split_multiwait_drains(nc)
    return nc


_CACHED = {}


def _get_nc():
    if "nc" not in _CACHED:
        _CACHED["nc"] = build_nc()
        _CACHED["convw"] = _build_convw()
    return _CACHED["nc"], _CACHED["convw"]


def combine_acc(acc_tiles):
    """acc_tiles: list of [128, 64] f32 -> scalar loss (f32)."""
    ng = 2 * B_PER_CORE * len(GROUPS)
    s_wlse = s_wpk = 0.0
    for a in acc_tiles:
        a = a.astype(np.float64)
        s_wlse += a[:, 0:ng].sum()
        s_wpk += a[:, 32:32 + ng].sum()
    n = 16 * H * W
    loss = (s_wlse - s_wpk) / n
    return np.float32(loss)


def kernel(pred, target):
    nc, convw = _get_nc()
    n_cores = 8
    in_maps = []
    for i in range(n_cores):
        in_maps.append({
            "pred": np.ascontiguousarray(pred[2 * i:2 * i + 2]),
            "target": np.ascontiguousarray(target[2 * i:2 * i + 2]),
            "convw": convw,
        })
    res = bass_utils.run_bass_kernel_spmd(nc, in_maps,
                                          core_ids=list(range(n_cores)))
    return combine_acc([r["acc"] for r in res.results])

